# revision 48
# baseline (speedup 1.0000x reference)
"""Trainium2 (8-core) kernel for a GQA attention layer with rotary embeddings.

Reference computation (N=2048 tokens, D=1024, H=16 q-heads, KV=4 kv-heads, HD=64):
    xq = rope(x @ Wq), xk = rope(x @ Wk), xv = x @ Wv
    out = softmax(xq xk^T / sqrt(HD)) @ xv   (full attention, GQA)
    return out @ Wo

Sharding: heads across the 8 cores (2 q-heads + their shared kv-head per core).
Attention runs per-core in a transposed (S^T) layout so no on-chip P transpose
is needed; softmax exp is split between the Scalar engine (table exp) and a
custom fused DVE op (cubic^4 approximation, valid because scores are
range-bounded).

Pipeline per core:
  x streams in token-block major; KV projections chase the DMA, then Q
  projections, then 4 attention rounds (one 512-token block each). After each
  round a small AllToAll ships a 64-token stripe of every core's output chunk,
  overlapping the next round's compute; the last round's exchange is split per
  head so the final collective is fired as early and as small as possible. The
  output projection for this core's 256 received tokens is slotted into PE
  gaps (first half after round 2, second half at the end).

All DMAs are issued on the HWDGE queues (Sync/Scalar) — the SWDGE (gpsimd)
path serializes at ~5us per transfer on this platform. The gpsimd queue
carries only the collective triggers so they fire with no queueing delay.

The host reassembles the striped output: core j's row (64t + i) is global
token (512t + 64j + i).
"""

import numpy as np
import ml_dtypes

import concourse.bass as bass
import concourse.tile as tile
from concourse import bacc, mybir
from concourse.tile_rust import add_dep_helper
from concourse.bass_utils import run_bass_kernel_spmd
from concourse.masks import make_identity

bf16 = ml_dtypes.bfloat16
BF16 = mybir.dt.bfloat16
F32 = mybir.dt.float32

N, D, H, KV, HD = 2048, 1024, 16, 4, 64
NC = 8
HPC = H // NC            # q-heads per core = 2
TOK = N // NC            # output token slice per core = 256
NKB = N // 128           # 16 k-blocks of 128 tokens
VW = 128                 # [ones (64 cols) | V (64 cols)] per k-block: the ones
                         # columns make the PV matmul emit 64 sum rows,
                         # a free partition-broadcast for the normalize step
NCHUNK = D // 128        # 8 contraction chunks for the projections

# exp(4y) ~= (1 + c1 y + c2 y^2 + c3 y^3)^4 minimax-fit on y in [-0.7, 0.7];
# scores/32 land in [-0.65, 0.65]. Max relative error ~0.65%.
EXP_C1, EXP_C2, EXP_C3 = 1.00305985, 0.51686418, 0.16136205

# which kbp iterations (per qb) run exp on the DVE instead of ACT
DVE_KBP = (1, 4, 6)


# ---------------------------------------------------------------- custom DVE op
def _register_exp_op():
    import concourse.dve_ops as dve_ops_mod
    from concourse.dve_spec import Spec, Src0, C0, C1, C2, One, sq, lower
    from concourse.dve_uop import DveOpSpec

    name = "EXP_CUBIC_POW4_ANT"
    for op in dve_ops_mod.OPS:
        if op.name == name:
            return op

    y = Src0
    p = ((C2 * y + C1) * y + C0) * y + One
    body = sq(sq(p))

    def ref(in0, in1, s0, s1, imm2):
        pp = ((imm2 * in0 + s1) * in0 + s0) * in0 + 1.0
        return (pp * pp) * (pp * pp)

    spec = Spec(body=body, reference=ref)
    row = dve_ops_mod._CUSTOM_DVE_ROW_BASE + len(dve_ops_mod.OPS)
    shas = {}
    for ver in ("v3", "v4"):
        try:
            uops = lower(spec, ver=ver)
            shas[ver] = DveOpSpec(name=name, opcode=row, uops=uops, rd1_en=False).sha(
                ver
            )
        except Exception:
            pass
    op = dve_ops_mod.DveOp(name, spec, subdim=False, uops_sha=shas)
    dve_ops_mod.OPS.append(op)
    dve_ops_mod.CUSTOM_DVE_SPECS[name] = spec
    dve_ops_mod._SUB_OPCODE_FOR_NAME[name] = row
    return op


EXP_OP = _register_exp_op()


# ---------------------------------------------------------------- device kernel
def _build_nc():
    nc = bacc.Bacc(
        "TRN2", target_bir_lowering=False, debug=False, num_devices=NC
    )
    xt = nc.dram_tensor(
        "xt", [4, 128, NCHUNK, 512], BF16, kind="ExternalInput"
    ).ap()
    wq = nc.dram_tensor("wq", [128, NCHUNK, 128], BF16, kind="ExternalInput").ap()
    wkv = nc.dram_tensor("wkv", [128, NCHUNK, 128], BF16, kind="ExternalInput").ap()
    wo = nc.dram_tensor("wo", [128, NCHUNK, D], BF16, kind="ExternalInput").ap()
    cosf = nc.dram_tensor("cosf", [128, N], BF16, kind="ExternalInput").ap()
    sinf = nc.dram_tensor("sinf", [128, N], BF16, kind="ExternalInput").ap()
    out = nc.dram_tensor("out", [TOK, D], F32, kind="ExternalOutput").ap()

    with tile.TileContext(nc) as tc:
        _emit(nc, tc, xt, wq, wkv, wo, cosf, sinf, out)
    nc.compile()
    return nc


def _emit(nc, tc, xt, wq, wkv, wo, cosf, sinf, out):
    fexp = mybir.ActivationFunctionType.Exp

    with (
        tc.tile_pool(name="persist", bufs=1) as pp,
        tc.tile_pool(name="work", bufs=3) as wp,
        tc.tile_pool(name="ppool", bufs=4) as ppool,
        tc.tile_pool(name="ps_stage", bufs=3, space="PSUM") as ps_stage,
        tc.tile_pool(name="ps_small", bufs=2, space="PSUM") as ps_small,
        tc.tile_pool(name="dram", bufs=1, space="DRAM") as dram,
    ):
        # ---- persistent SBUF tensors
        xall = pp.tile([128, 4, NCHUNK, 512], BF16, tag="xall")
        wq_sb = pp.tile([128, NCHUNK, 128], BF16, tag="wq")
        wkv_sb = pp.tile([128, NCHUNK, 128], BF16, tag="wkv")
        wo_sb = pp.tile([128, NCHUNK, D], BF16, tag="wo")
        cos_sb = pp.tile([128, N], BF16, tag="cos")
        sin_sb = pp.tile([128, N], BF16, tag="sin")
        # q^T per token block (separate tiles so round t never waits on a
        # later block's rope chain via tile-granular dependency tracking)
        qtd = [
            pp.tile([128, HPC * 512], BF16, tag=f"qtd{t}", name=f"qtd{t}")
            for t in range(4)
        ]
        ktd = pp.tile([128, N], BF16, tag="ktd")        # k^T duplicated rows
        vp = pp.tile([128, NKB, VW], BF16, tag="vp")    # [ones | V] per k-block
        ofin = pp.tile([128, N], BF16, tag="ofin")      # normalized attn out^T
        og_a = pp.tile([128, NCHUNK, 128], BF16, tag="og_a")  # rounds 0-1 gather
        og_b = pp.tile([128, NCHUNK, 128], BF16, tag="og_b")  # rounds 2-3 gather
        ident = pp.tile([128, 128], BF16, tag="ident")

        a2a_in = [
            dram.tile([NC, 128, 64], BF16, tag=f"ain{t}", name=f"ain{t}")
            for t in range(3)
        ]
        a2a_out = [
            dram.tile([NC, 128, 64], BF16, tag=f"aout{t}", name=f"aout{t}")
            for t in range(3)
        ]
        # round 3 exchanges per head (half the rows), so the last collective
        # fires half a round early and the final one is as small as possible
        a2a_in3 = [
            dram.tile([NC, 64, 64], BF16, tag=f"ain3{h}", name=f"ain3{h}")
            for h in range(2)
        ]
        a2a_out3 = [
            dram.tile([NC, 64, 64], BF16, tag=f"aout3{h}", name=f"aout3{h}")
            for h in range(2)
        ]

        # ---- loads: x on the Sync queue, everything else on the Scalar queue.
        # xt is host-staged as [tb, p, c, t] so each block is one fully
        # contiguous 1MB HBM read. Concurrent DMA queues fair-share HBM, so
        # explicit dep edges serialize the stream in need-order: each transfer
        # gets full bandwidth and the first-needed bytes land first.
        x_dma = []

        def dma_x(tb):
            inst = nc.sync.dma_start(xall[:, tb, :, :], xt[tb])
            if tb > 0:
                add_dep_helper(
                    inst.ins, x_dma[tb - 1].ins, reason="serialize x stream"
                )
            x_dma.append(inst)

        # x block 0 arrives in two halves so the first projection matmuls can
        # start after 0.5MB instead of 1MB (contiguous 4KB rows per half)
        x0a = nc.sync.dma_start(xall[:, 0, 0:4, :], xt[0, :, 0:4, :])
        x0b = nc.sync.dma_start(xall[:, 0, 4:8, :], xt[0, :, 4:8, :])
        add_dep_helper(x0b.ins, x0a.ins, reason="serialize x stream")
        x_dma.append(x0b)
        d_wkv = nc.scalar.dma_start(wkv_sb[:], wkv)
        d_wq = nc.scalar.dma_start(wq_sb[:], wq)
        add_dep_helper(d_wq.ins, d_wkv.ins, reason="weights after wkv")
        d_cos = nc.scalar.dma_start(cos_sb[:], cosf)
        d_sin = nc.scalar.dma_start(sin_sb[:], sinf)
        make_identity(nc, ident[:])
        nc.vector.memset(vp[:], 1.0)

        def collective(ain, aout):
            nc.gpsimd.collective_compute(
                "AllToAll",
                mybir.AluOpType.bypass,
                replica_groups=[list(range(NC))],
                ins=[ain.opt()],
                outs=[aout.opt()],
            )

        def proj(tb, w_sb, tag):
            ps = ps_small.tile([128, 512], F32, tag="small")
            for c in range(NCHUNK):
                nc.tensor.matmul(
                    ps[:], w_sb[:, c, :], xall[:, tb, c, :],
                    start=(c == 0), stop=(c == NCHUNK - 1),
                )
            sb = wp.tile([128, 512], BF16, tag=tag)
            nc.scalar.copy(sb[:], ps[:])
            return sb

        def rope_k(tb, kvsb):
            ts = slice(tb * 512, (tb + 1) * 512)
            ksw = wp.tile([64, 512], BF16, tag="ksw")
            nc.scalar.dma_start(ksw[0:32, :], kvsb[32:64, :])
            nc.scalar.dma_start(ksw[32:64, :], kvsb[0:32, :])
            t1k = wp.tile([64, 512], BF16, tag="ropet1k")
            nc.vector.tensor_mul(t1k[:], kvsb[0:64, :], cos_sb[0:64, ts])
            t2k = wp.tile([64, 512], BF16, tag="ropet2k")
            nc.vector.tensor_mul(t2k[:], ksw[:], sin_sb[0:64, ts])
            kr = wp.tile([64, 512], BF16, tag="krot")
            nc.vector.tensor_add(kr[:], t1k[:], t2k[:])
            nc.scalar.dma_start(ktd[0:64, ts], kr[:])
            nc.scalar.dma_start(ktd[64:128, ts], kr[:])

        def vtrans(tb, kvsb):
            # V natural: transpose v^T (rows 64-127) in 128-col blocks
            for j in range(4):
                kb = tb * 4 + j
                vt = ps_small.tile([128, 64], BF16, tag="small")
                nc.tensor.transpose(
                    vt[:],
                    kvsb[64:128, j * 128 : (j + 1) * 128],
                    ident[64:128, 64:128],
                )
                nc.vector.tensor_copy(vp[:, kb, 64 : 64 + HD], vt[:])

        def rope_q(tb, qsb):
            # q block 0 runs during the KV phase (scalar queue is free there);
            # later blocks run mid-rounds where the sync queue has the room
            eng = nc.scalar if tb == 0 else nc.sync
            ts = slice(tb * 512, (tb + 1) * 512)
            qsw = wp.tile([128, 512], BF16, tag="qsw")
            for b in (0, 64):
                eng.dma_start(qsw[b : b + 32, :], qsb[b + 32 : b + 64, :])
                eng.dma_start(qsw[b + 32 : b + 64, :], qsb[b : b + 32, :])
            t1 = wp.tile([128, 512], BF16, tag="ropet1")
            nc.vector.tensor_mul(t1[:], qsb[:], cos_sb[:, ts])
            t2 = wp.tile([128, 512], BF16, tag="ropet2")
            nc.vector.tensor_mul(t2[:], qsw[:], sin_sb[:, ts])
            qr = wp.tile([128, 512], BF16, tag="qrot")
            nc.vector.tensor_add(qr[:], t1[:], t2[:])
            for h in range(HPC):
                dst = slice(h * 512, (h + 1) * 512)
                src = qr[64 * h : 64 * h + 64, :]
                eng.dma_start(qtd[tb][0:64, dst], src)
                eng.dma_start(qtd[tb][64:128, dst], src)

        # KV projections chase the x stream; q block 0 is projected right after
        # KV block 0 so its rope chain completes long before round 0. V
        # transposes run after all KV blocks so the PE never waits on a
        # PSUM->SBUF copy.
        kvsbs = []
        for tb in range(4):
            kvsbs.append(proj(tb, wkv_sb, "kvsb"))
            rope_k(tb, kvsbs[tb])
            if tb == 0:
                rope_q(0, proj(0, wq_sb, "qsb"))
            if tb < 3:
                dma_x(tb + 1)
        # schedule the rope tables and Wo behind the x stream (edges can be
        # added once both instructions exist; emission order above already
        # guarantees readers see the writes)
        add_dep_helper(d_cos.ins, x_dma[1].ins, reason="cos after x1")
        add_dep_helper(d_sin.ins, d_cos.ins, reason="sin after cos")
        d_wo = nc.scalar.dma_start(wo_sb[:], wo)
        add_dep_helper(d_wo.ins, x_dma[3].ins, reason="wo after x3")

        def qproj(tb):
            rope_q(tb, proj(tb, wq_sb, "qsb"))

        # ---- attention rounds: round t covers token block [512t, 512t+512)
        def attn_qb(t, h, flush=None):
            qs = slice(h * 512, (h + 1) * 512)
            acc = ps_small.tile([128, 512], F32, tag="small")

            # software-pipelined emission: scores(kbp+1) is queued on the PE
            # before PV(kbp), so the PE never waits on the exp engines
            def stage(kbp):
                kb0, kb1 = 2 * kbp, 2 * kbp + 1
                st = ps_stage.tile([128, 1024], F32, tag="stage")
                nc.tensor.matmul(
                    st[:, 0:512],
                    ktd[0:64, kb0 * 128 : (kb0 + 1) * 128],
                    qtd[t][0:64, qs],
                    start=True, stop=True,
                )
                nc.tensor.matmul(
                    st[:, 512:1024],
                    ktd[64:128, kb1 * 128 : (kb1 + 1) * 128],
                    qtd[t][64:128, qs],
                    start=True, stop=True,
                )
                pt = ppool.tile([128, 1024], BF16, tag="pt")
                if kbp in DVE_KBP:
                    nc.vector._custom_dve(
                        EXP_OP, out=pt[:], in0=st[:],
                        s0=EXP_C1, s1=EXP_C2, imm2=EXP_C3,
                    )
                else:
                    nc.scalar.activation(pt[:], st[:], fexp, scale=4.0)
                return pt

            def pv(kbp, pt):
                kb0, kb1 = 2 * kbp, 2 * kbp + 1
                nc.tensor.matmul(
                    acc[:],
                    vp[:, kb0, :],
                    pt[:, 0:512],
                    start=(kbp == 0), stop=False,
                )
                nc.tensor.matmul(
                    acc[:],
                    vp[:, kb1, :],
                    pt[:, 512:1024],
                    start=False, stop=(kbp == NKB // 2 - 1),
                )

            pt_prev = stage(0)
            for kbp in range(1, NKB // 2):
                pt_cur = stage(kbp)
                if kbp == 2 and flush is not None:
                    # deferred normalize of the previous qb: emitted here so
                    # this qb's first exps reach the ACT/DVE queues ahead of it
                    flush()
                pv(kbp - 1, pt_prev)
                pt_prev = pt_cur
            pv(NKB // 2 - 1, pt_prev)

            def norm():
                # normalize: copy acc to SBUF, realign the out-half to
                # partition base 0 by DMA, then all ops are base-aligned
                asb = wp.tile([128, 512], F32, tag="asb")
                nc.scalar.copy(asb[:], acc[:])
                obuf = wp.tile([64, 512], F32, tag="obuf")
                nc.scalar.dma_start(obuf[:], asb[64:128, :])
                rs = wp.tile([64, 512], F32, tag="rsum")
                nc.vector.reciprocal_approx_fast(rs[:], asb[0:64, :])
                ot = wp.tile([64, 512], BF16, tag="onorm")
                nc.vector.tensor_mul(ot[:], obuf[:], rs[:])
                nc.scalar.dma_start(
                    ofin[64 * h : 64 * h + 64, 512 * t : 512 * (t + 1)], ot[:]
                )

            return norm

        def oproj_half(mm, og):
            # tokens [128*mm, 128*mm+128) of my 256-token slice
            for n_ in range(2):
                po = ps_small.tile([128, 512], F32, tag="small")
                for c in range(NCHUNK):
                    nc.tensor.matmul(
                        po[:],
                        og[:, c, :],
                        wo_sb[:, c, n_ * 512 : (n_ + 1) * 512],
                        start=(c == 0), stop=(c == NCHUNK - 1),
                    )
                osb = wp.tile([128, 512], F32, tag="osb")
                nc.scalar.copy(osb[:], po[:])
                nc.sync.dma_start(
                    out[mm * 128 : (mm + 1) * 128, n_ * 512 : (n_ + 1) * 512],
                    osb[:],
                )

        # Q projections interleave with the rounds: round t only needs its own
        # token block's q, so the PE streams Q(t+2) while round t runs and no
        # round ever binds to a later block's rope chain. og receives are
        # emitted AFTER the next round's a2a_in on the same (Sync) queue so a
        # pending collective never blocks the next trigger.
        og_recv = []

        def recv(t):
            og = og_a if t < 2 else og_b
            nc.sync.dma_start(
                og[:, :, 64 * (t % 2) : 64 * (t % 2) + 64],
                a2a_out[t].rearrange("c p i -> p c i"),
            )

        def exchange(t):
            # AllToAll: shard j = my rows for tokens [512t + 64j, +64)
            nc.sync.dma_start(
                a2a_in[t].rearrange("j p i -> p j i"),
                ofin[:, 512 * t : 512 * (t + 1)],
            )
            if t > 0:
                recv(t - 1)
            collective(a2a_in[t], a2a_out[t])

        for tb in range(4):
            vtrans(tb, kvsbs[tb])
        qproj(1)
        nflush = attn_qb(0, 0)
        qproj(2)
        nflush = attn_qb(0, 1, flush=nflush)
        nflush()
        exchange(0)
        nflush = attn_qb(1, 0)
        qproj(3)
        nflush = attn_qb(1, 1, flush=nflush)
        nflush()
        exchange(1)
        nflush = attn_qb(2, 0)
        nflush = attn_qb(2, 1, flush=nflush)
        nflush()
        exchange(2)
        # round 3, head by head; first output-projection half in between
        nflush = attn_qb(3, 0)
        nflush()
        nc.sync.dma_start(
            a2a_in3[0].rearrange("j p i -> p j i"), ofin[0:64, 1536:2048]
        )
        recv(2)
        collective(a2a_in3[0], a2a_out3[0])
        # keep-warm dummies queued on the CC engine right behind #3a: the ncfw
        # firmware goes to sleep the moment its queue drains and takes ~11us
        # to wake, so these bridge the gap until round 3 head 1's trigger
        dmy = [
            dram.tile([NC, 512], BF16, tag=f"dmy{i}", name=f"dmy{i}")
            for i in range(4)
        ]
        collective(dmy[0], dmy[1])
        collective(dmy[2], dmy[3])
        oproj_half(0, og_a)
        nflush = attn_qb(3, 1)
        nflush()
        nc.sync.dma_start(
            a2a_in3[1].rearrange("j p i -> p j i"), ofin[64:128, 1536:2048]
        )
        nc.scalar.dma_start(
            og_b[0:64, :, 64:128], a2a_out3[0].rearrange("c p i -> p c i")
        )
        collective(a2a_in3[1], a2a_out3[1])
        # output projection for tokens 128:256, contraction split by peer-row
        # half: the head0-rows half depends only on cc#2 + #3a and runs while
        # #3b is still in flight; only the head1-rows half waits for #3b.
        po_f = []
        for n_ in range(2):
            po = ps_small.tile([128, 512], F32, tag="small", name=f"po_f{n_}")
            for c in range(NCHUNK):
                nc.tensor.matmul(
                    po[:],
                    og_b[0:64, c, :],
                    wo_sb[0:64, c, n_ * 512 : (n_ + 1) * 512],
                    start=(c == 0), stop=False,
                )
            po_f.append(po)
        nc.scalar.dma_start(
            og_b[64:128, :, 64:128], a2a_out3[1].rearrange("c p i -> p c i")
        )
        for n_ in range(2):
            for c in range(NCHUNK):
                nc.tensor.matmul(
                    po_f[n_][:],
                    og_b[64:128, c, :],
                    wo_sb[64:128, c, n_ * 512 : (n_ + 1) * 512],
                    start=False, stop=(c == NCHUNK - 1),
                )
            osb = wp.tile([128, 512], F32, tag="osb")
            nc.scalar.copy(osb[:], po_f[n_][:])
            nc.sync.dma_start(
                out[128:256, n_ * 512 : (n_ + 1) * 512], osb[:]
            )


_NC_CACHE = None


def _get_nc():
    global _NC_CACHE
    if _NC_CACHE is None:
        _NC_CACHE = _build_nc()
    return _NC_CACHE


# ---------------------------------------------------------------- host wrapper
_ROPE_PERM = np.concatenate([np.arange(0, HD, 2), np.arange(1, HD, 2)])


def _chunked(w):
    """(D, F) -> (128, D//128, F) so [p, c, f] = w[128c+p, f]."""
    return np.ascontiguousarray(
        w.reshape(D // 128, 128, -1).transpose(1, 0, 2)
    )


def _prep_inputs(x, freqs_cos, freqs_sin, Wq, Wk, Wv, Wo):
    x = np.asarray(x, np.float32)
    Wq = np.asarray(Wq, np.float32)
    Wk = np.asarray(Wk, np.float32)
    Wv = np.asarray(Wv, np.float32)
    Wo = np.asarray(Wo, np.float32)
    cos = np.asarray(freqs_cos, np.float32)
    sin = np.asarray(freqs_sin, np.float32)

    # [tb, p, c, t]: element = x[512*tb + t, 128*c + p]; each tb block is one
    # contiguous 1MB HBM read on the device
    xt = np.ascontiguousarray(
        x.reshape(4, 512, NCHUNK, 128).transpose(0, 3, 2, 1)
    ).astype(bf16)
    cosf = np.tile(cos.T, (4, 1)).astype(bf16)
    # signed sin table matching the [real(32); imag(32)] row blocks:
    # q' = q*cos + swap(q)*[-s; +s]
    sinf = np.tile(np.concatenate([-sin.T, sin.T], axis=0), (2, 1)).astype(bf16)
    wo_dev = _chunked(Wo).astype(bf16)

    in_maps = []
    for r in range(NC):
        h0, h1 = 2 * r, 2 * r + 1
        g = r // 2
        # q pre-scaled by 1/32: folds the 1/sqrt(HD)=1/8 softmax scale and the
        # /4 for the (cubic)^4 exp decomposition into the weights.
        wq_core = np.concatenate(
            [
                Wq[:, 64 * h0 + _ROPE_PERM],
                Wq[:, 64 * h1 + _ROPE_PERM],
            ],
            axis=1,
        ) * (1.0 / 32.0)
        wkv_core = np.concatenate(
            [Wk[:, 64 * g + _ROPE_PERM], Wv[:, 64 * g : 64 * g + HD]], axis=1
        )
        in_maps.append(
            {
                "xt": xt,
                "wq": _chunked(wq_core).astype(bf16),
                "wkv": _chunked(wkv_core).astype(bf16),
                "wo": wo_dev,
                "cosf": cosf,
                "sinf": sinf,
            }
        )
    return in_maps


def _run(inputs, trace=False, **spmd_kwargs):
    in_maps = _prep_inputs(**inputs)
    nc = _get_nc()
    res = run_bass_kernel_spmd(
        nc, in_maps, core_ids=list(range(NC)), trace=trace, **spmd_kwargs
    )
    # core j's local row (64t + i) is global token (512t + 64j + i)
    allres = np.stack([res.results[r]["out"] for r in range(NC)])  # [8, 256, D]
    full = (
        allres.reshape(NC, 4, 64, D).transpose(1, 0, 2, 3).reshape(N, D)
    )
    return full.astype(np.float32), res


def kernel(**inputs):
    out, _ = _run(inputs, trace=False)
    return out


# revision 49
# speedup vs baseline: 1.1684x; 1.1684x over previous
"""Trainium2 (8-core) kernel for a GQA attention layer with rotary embeddings.

Reference computation (N=2048 tokens, D=1024, H=16 q-heads, KV=4 kv-heads, HD=64):
    xq = rope(x @ Wq), xk = rope(x @ Wk), xv = x @ Wv
    out = softmax(xq xk^T / sqrt(HD)) @ xv   (full attention, GQA)
    return out @ Wo

Sharding: heads across the 8 cores (2 q-heads + their shared kv-head per core).
Attention runs per-core in a transposed (S^T) layout so no on-chip P transpose
is needed; softmax exp is split between the Scalar engine (table exp) and a
custom fused DVE op (cubic^4 approximation, valid because scores are
range-bounded).

Pipeline per core:
  x streams in token-block major; KV projections chase the DMA, then Q
  projections, then 4 attention rounds (one 512-token block each). After each
  round a small AllToAll ships a 64-token stripe of every core's output chunk,
  overlapping the next round's compute; the last round's exchange is split per
  head so the final collective is fired as early and as small as possible. The
  output projection for this core's 256 received tokens is slotted into PE
  gaps (first half after round 2, second half at the end).

All DMAs are issued on the HWDGE queues (Sync/Scalar) — the SWDGE (gpsimd)
path serializes at ~5us per transfer on this platform. The gpsimd queue
carries only the collective triggers so they fire with no queueing delay.

The host reassembles the striped output: core j's row (64t + i) is global
token (512t + 64j + i).
"""

import numpy as np
import ml_dtypes

import concourse.bass as bass
import concourse.tile as tile
from concourse import bacc, mybir
from concourse.tile_rust import add_dep_helper
from concourse.bass_utils import run_bass_kernel_spmd
from concourse.masks import make_identity

bf16 = ml_dtypes.bfloat16
BF16 = mybir.dt.bfloat16
F32 = mybir.dt.float32

N, D, H, KV, HD = 2048, 1024, 16, 4, 64
NC = 8
HPC = H // NC            # q-heads per core = 2
TOK = N // NC            # output token slice per core = 256
NKB = N // 128           # 16 k-blocks of 128 tokens
VW = 128                 # [ones (64 cols) | V (64 cols)] per k-block: the ones
                         # columns make the PV matmul emit 64 sum rows,
                         # a free partition-broadcast for the normalize step
NCHUNK = D // 128        # 8 contraction chunks for the projections

# exp(4y) ~= (1 + c1 y + c2 y^2 + c3 y^3)^4 minimax-fit on y in [-0.7, 0.7];
# scores/32 land in [-0.65, 0.65]. Max relative error ~0.65%.
EXP_C1, EXP_C2, EXP_C3 = 1.00305985, 0.51686418, 0.16136205

# which kbp iterations (per qb) run exp on the DVE instead of ACT
DVE_KBP = (1, 4, 6)


# ---------------------------------------------------------------- custom DVE op
def _register_exp_op():
    import concourse.dve_ops as dve_ops_mod
    from concourse.dve_spec import Spec, Src0, C0, C1, C2, One, sq, lower
    from concourse.dve_uop import DveOpSpec

    name = "EXP_CUBIC_POW4_ANT"
    for op in dve_ops_mod.OPS:
        if op.name == name:
            return op

    y = Src0
    p = ((C2 * y + C1) * y + C0) * y + One
    body = sq(sq(p))

    def ref(in0, in1, s0, s1, imm2):
        pp = ((imm2 * in0 + s1) * in0 + s0) * in0 + 1.0
        return (pp * pp) * (pp * pp)

    spec = Spec(body=body, reference=ref)
    row = dve_ops_mod._CUSTOM_DVE_ROW_BASE + len(dve_ops_mod.OPS)
    shas = {}
    for ver in ("v3", "v4"):
        try:
            uops = lower(spec, ver=ver)
            shas[ver] = DveOpSpec(name=name, opcode=row, uops=uops, rd1_en=False).sha(
                ver
            )
        except Exception:
            pass
    op = dve_ops_mod.DveOp(name, spec, subdim=False, uops_sha=shas)
    dve_ops_mod.OPS.append(op)
    dve_ops_mod.CUSTOM_DVE_SPECS[name] = spec
    dve_ops_mod._SUB_OPCODE_FOR_NAME[name] = row
    return op


EXP_OP = _register_exp_op()


# ---------------------------------------------------------------- device kernel
def _build_nc():
    nc = bacc.Bacc(
        "TRN2", target_bir_lowering=False, debug=False, num_devices=NC
    )
    xt = nc.dram_tensor(
        "xt", [4, 128, NCHUNK, 512], BF16, kind="ExternalInput"
    ).ap()
    wq = nc.dram_tensor("wq", [128, NCHUNK, 128], BF16, kind="ExternalInput").ap()
    wkv = nc.dram_tensor("wkv", [128, NCHUNK, 128], BF16, kind="ExternalInput").ap()
    wo = nc.dram_tensor("wo", [128, NCHUNK, D], BF16, kind="ExternalInput").ap()
    cosf = nc.dram_tensor("cosf", [128, N], BF16, kind="ExternalInput").ap()
    sinf = nc.dram_tensor("sinf", [128, N], BF16, kind="ExternalInput").ap()
    out = nc.dram_tensor("out", [TOK, D], F32, kind="ExternalOutput").ap()

    with tile.TileContext(nc) as tc:
        _emit(nc, tc, xt, wq, wkv, wo, cosf, sinf, out)
    nc.compile()
    return nc


def _emit(nc, tc, xt, wq, wkv, wo, cosf, sinf, out):
    fexp = mybir.ActivationFunctionType.Exp

    with (
        tc.tile_pool(name="persist", bufs=1) as pp,
        tc.tile_pool(name="work", bufs=3) as wp,
        tc.tile_pool(name="ppool", bufs=4) as ppool,
        tc.tile_pool(name="ps_stage", bufs=3, space="PSUM") as ps_stage,
        tc.tile_pool(name="ps_small", bufs=2, space="PSUM") as ps_small,
        tc.tile_pool(name="dram", bufs=1, space="DRAM") as dram,
    ):
        # ---- persistent SBUF tensors
        xall = pp.tile([128, 4, NCHUNK, 512], BF16, tag="xall")
        wq_sb = pp.tile([128, NCHUNK, 128], BF16, tag="wq")
        wkv_sb = pp.tile([128, NCHUNK, 128], BF16, tag="wkv")
        wo_sb = pp.tile([128, NCHUNK, D], BF16, tag="wo")
        cos_sb = pp.tile([128, N], BF16, tag="cos")
        sin_sb = pp.tile([128, N], BF16, tag="sin")
        # q^T per token block (separate tiles so round t never waits on a
        # later block's rope chain via tile-granular dependency tracking)
        qtd = [
            pp.tile([128, HPC * 512], BF16, tag=f"qtd{t}", name=f"qtd{t}")
            for t in range(4)
        ]
        ktd = pp.tile([128, N], BF16, tag="ktd")        # k^T duplicated rows
        vp = pp.tile([128, NKB, VW], BF16, tag="vp")    # [ones | V] per k-block
        ofin = pp.tile([128, N], BF16, tag="ofin")      # normalized attn out^T
        og_a = pp.tile([128, NCHUNK, 128], BF16, tag="og_a")  # rounds 0-1 gather
        og_b = pp.tile([128, NCHUNK, 128], BF16, tag="og_b")  # rounds 2-3 gather
        ident = pp.tile([128, 128], BF16, tag="ident")

        a2a_in = [
            dram.tile([NC, 128, 64], BF16, tag=f"ain{t}", name=f"ain{t}")
            for t in range(3)
        ]
        a2a_out = [
            dram.tile([NC, 128, 64], BF16, tag=f"aout{t}", name=f"aout{t}")
            for t in range(3)
        ]
        # round 3 exchanges per head (half the rows), so the last collective
        # fires half a round early and the final one is as small as possible
        a2a_in3 = [
            dram.tile([NC, 64, 64], BF16, tag=f"ain3{h}", name=f"ain3{h}")
            for h in range(2)
        ]
        a2a_out3 = [
            dram.tile([NC, 64, 64], BF16, tag=f"aout3{h}", name=f"aout3{h}")
            for h in range(2)
        ]

        # ---- loads: x on the Sync queue, everything else on the Scalar queue.
        # xt is host-staged as [tb, p, c, t] so each block is one fully
        # contiguous 1MB HBM read. Concurrent DMA queues fair-share HBM, so
        # explicit dep edges serialize the stream in need-order: each transfer
        # gets full bandwidth and the first-needed bytes land first.
        x_dma = []

        def dma_x(tb):
            inst = nc.sync.dma_start(xall[:, tb, :, :], xt[tb])
            if tb > 0:
                add_dep_helper(
                    inst.ins, x_dma[tb - 1].ins, reason="serialize x stream"
                )
            x_dma.append(inst)

        # x block 0 arrives in two halves so the first projection matmuls can
        # start after 0.5MB instead of 1MB (contiguous 4KB rows per half)
        x0a = nc.sync.dma_start(xall[:, 0, 0:4, :], xt[0, :, 0:4, :])
        x0b = nc.sync.dma_start(xall[:, 0, 4:8, :], xt[0, :, 4:8, :])
        add_dep_helper(x0b.ins, x0a.ins, reason="serialize x stream")
        x_dma.append(x0b)
        d_wkv = nc.scalar.dma_start(wkv_sb[:], wkv)
        d_wq = nc.scalar.dma_start(wq_sb[:], wq)
        add_dep_helper(d_wq.ins, d_wkv.ins, reason="weights after wkv")
        d_cos = nc.scalar.dma_start(cos_sb[:], cosf)
        d_sin = nc.scalar.dma_start(sin_sb[:], sinf)
        make_identity(nc, ident[:])
        nc.vector.memset(vp[:], 1.0)

        def collective(ain, aout):
            nc.gpsimd.collective_compute(
                "AllToAll",
                mybir.AluOpType.bypass,
                replica_groups=[list(range(NC))],
                ins=[ain.opt()],
                outs=[aout.opt()],
            )

        def proj(tb, w_sb, tag):
            ps = ps_small.tile([128, 512], F32, tag="small")
            for c in range(NCHUNK):
                nc.tensor.matmul(
                    ps[:], w_sb[:, c, :], xall[:, tb, c, :],
                    start=(c == 0), stop=(c == NCHUNK - 1),
                )
            sb = wp.tile([128, 512], BF16, tag=tag)
            nc.scalar.copy(sb[:], ps[:])
            return sb

        def rope_k(tb, kvsb):
            ts = slice(tb * 512, (tb + 1) * 512)
            ksw = wp.tile([64, 512], BF16, tag="ksw")
            nc.scalar.dma_start(ksw[0:32, :], kvsb[32:64, :])
            nc.scalar.dma_start(ksw[32:64, :], kvsb[0:32, :])
            t1k = wp.tile([64, 512], BF16, tag="ropet1k")
            nc.vector.tensor_mul(t1k[:], kvsb[0:64, :], cos_sb[0:64, ts])
            t2k = wp.tile([64, 512], BF16, tag="ropet2k")
            nc.vector.tensor_mul(t2k[:], ksw[:], sin_sb[0:64, ts])
            kr = wp.tile([64, 512], BF16, tag="krot")
            nc.vector.tensor_add(kr[:], t1k[:], t2k[:])
            nc.scalar.dma_start(ktd[0:64, ts], kr[:])
            nc.scalar.dma_start(ktd[64:128, ts], kr[:])

        def vtrans(tb, kvsb):
            # V natural: transpose v^T (rows 64-127) in 128-col blocks
            for j in range(4):
                kb = tb * 4 + j
                vt = ps_small.tile([128, 64], BF16, tag="small")
                nc.tensor.transpose(
                    vt[:],
                    kvsb[64:128, j * 128 : (j + 1) * 128],
                    ident[64:128, 64:128],
                )
                nc.vector.tensor_copy(vp[:, kb, 64 : 64 + HD], vt[:])

        def rope_q(tb, qsb):
            # q block 0 runs during the KV phase (scalar queue is free there);
            # later blocks run mid-rounds where the sync queue has the room
            eng = nc.scalar if tb == 0 else nc.sync
            ts = slice(tb * 512, (tb + 1) * 512)
            qsw = wp.tile([128, 512], BF16, tag="qsw")
            for b in (0, 64):
                eng.dma_start(qsw[b : b + 32, :], qsb[b + 32 : b + 64, :])
                eng.dma_start(qsw[b + 32 : b + 64, :], qsb[b : b + 32, :])
            t1 = wp.tile([128, 512], BF16, tag="ropet1")
            nc.vector.tensor_mul(t1[:], qsb[:], cos_sb[:, ts])
            t2 = wp.tile([128, 512], BF16, tag="ropet2")
            nc.vector.tensor_mul(t2[:], qsw[:], sin_sb[:, ts])
            qr = wp.tile([128, 512], BF16, tag="qrot")
            nc.vector.tensor_add(qr[:], t1[:], t2[:])
            for h in range(HPC):
                dst = slice(h * 512, (h + 1) * 512)
                src = qr[64 * h : 64 * h + 64, :]
                eng.dma_start(qtd[tb][0:64, dst], src)
                eng.dma_start(qtd[tb][64:128, dst], src)

        # KV projections chase the x stream; q block 0 is projected right after
        # KV block 0 so its rope chain completes long before round 0. V
        # transposes run after all KV blocks so the PE never waits on a
        # PSUM->SBUF copy.
        kvsbs = []
        for tb in range(4):
            kvsbs.append(proj(tb, wkv_sb, "kvsb"))
            rope_k(tb, kvsbs[tb])
            if tb == 0:
                rope_q(0, proj(0, wq_sb, "qsb"))
            if tb < 3:
                dma_x(tb + 1)
        # schedule the rope tables and Wo behind the x stream (edges can be
        # added once both instructions exist; emission order above already
        # guarantees readers see the writes)
        add_dep_helper(d_cos.ins, x_dma[1].ins, reason="cos after x1")
        add_dep_helper(d_sin.ins, d_cos.ins, reason="sin after cos")
        d_wo = nc.scalar.dma_start(wo_sb[:], wo)
        add_dep_helper(d_wo.ins, x_dma[3].ins, reason="wo after x3")

        def qproj(tb):
            rope_q(tb, proj(tb, wq_sb, "qsb"))

        # ---- attention rounds: round t covers token block [512t, 512t+512)
        def attn_qb(t, h, flush=None):
            qs = slice(h * 512, (h + 1) * 512)
            acc = ps_small.tile([128, 512], F32, tag="small")

            # software-pipelined emission: scores(kbp+1) is queued on the PE
            # before PV(kbp), so the PE never waits on the exp engines
            def stage(kbp):
                kb0, kb1 = 2 * kbp, 2 * kbp + 1
                st = ps_stage.tile([128, 1024], F32, tag="stage")
                nc.tensor.matmul(
                    st[:, 0:512],
                    ktd[0:64, kb0 * 128 : (kb0 + 1) * 128],
                    qtd[t][0:64, qs],
                    start=True, stop=True,
                )
                nc.tensor.matmul(
                    st[:, 512:1024],
                    ktd[64:128, kb1 * 128 : (kb1 + 1) * 128],
                    qtd[t][64:128, qs],
                    start=True, stop=True,
                )
                pt = ppool.tile([128, 1024], BF16, tag="pt")
                if kbp in DVE_KBP:
                    nc.vector._custom_dve(
                        EXP_OP, out=pt[:], in0=st[:],
                        s0=EXP_C1, s1=EXP_C2, imm2=EXP_C3,
                    )
                else:
                    nc.scalar.activation(pt[:], st[:], fexp, scale=4.0)
                return pt

            def pv(kbp, pt):
                kb0, kb1 = 2 * kbp, 2 * kbp + 1
                nc.tensor.matmul(
                    acc[:],
                    vp[:, kb0, :],
                    pt[:, 0:512],
                    start=(kbp == 0), stop=False,
                )
                nc.tensor.matmul(
                    acc[:],
                    vp[:, kb1, :],
                    pt[:, 512:1024],
                    start=False, stop=(kbp == NKB // 2 - 1),
                )

            pt_prev = stage(0)
            for kbp in range(1, NKB // 2):
                pt_cur = stage(kbp)
                if kbp == 2 and flush is not None:
                    # deferred normalize of the previous qb: emitted here so
                    # this qb's first exps reach the ACT/DVE queues ahead of it
                    flush()
                pv(kbp - 1, pt_prev)
                pt_prev = pt_cur
            pv(NKB // 2 - 1, pt_prev)

            def norm():
                # normalize: copy acc to SBUF, realign the out-half to
                # partition base 0 by DMA, then all ops are base-aligned
                asb = wp.tile([128, 512], F32, tag="asb")
                nc.scalar.copy(asb[:], acc[:])
                obuf = wp.tile([64, 512], F32, tag="obuf")
                nc.scalar.dma_start(obuf[:], asb[64:128, :])
                rs = wp.tile([64, 512], F32, tag="rsum")
                nc.vector.reciprocal_approx_fast(rs[:], asb[0:64, :])
                ot = wp.tile([64, 512], BF16, tag="onorm")
                nc.vector.tensor_mul(ot[:], obuf[:], rs[:])
                nc.scalar.dma_start(
                    ofin[64 * h : 64 * h + 64, 512 * t : 512 * (t + 1)], ot[:]
                )

            return norm

        def oproj_half(mm, og):
            # tokens [128*mm, 128*mm+128) of my 256-token slice
            for n_ in range(2):
                po = ps_small.tile([128, 512], F32, tag="small")
                for c in range(NCHUNK):
                    nc.tensor.matmul(
                        po[:],
                        og[:, c, :],
                        wo_sb[:, c, n_ * 512 : (n_ + 1) * 512],
                        start=(c == 0), stop=(c == NCHUNK - 1),
                    )
                osb = wp.tile([128, 512], F32, tag="osb")
                nc.scalar.copy(osb[:], po[:])
                nc.sync.dma_start(
                    out[mm * 128 : (mm + 1) * 128, n_ * 512 : (n_ + 1) * 512],
                    osb[:],
                )

        # Q projections interleave with the rounds: round t only needs its own
        # token block's q, so the PE streams Q(t+2) while round t runs and no
        # round ever binds to a later block's rope chain. og receives are
        # emitted AFTER the next round's a2a_in on the same (Sync) queue so a
        # pending collective never blocks the next trigger.
        og_recv = []

        def recv(t):
            og = og_a if t < 2 else og_b
            nc.sync.dma_start(
                og[:, :, 64 * (t % 2) : 64 * (t % 2) + 64],
                a2a_out[t].rearrange("c p i -> p c i"),
            )

        def exchange(t):
            # AllToAll: shard j = my rows for tokens [512t + 64j, +64)
            nc.sync.dma_start(
                a2a_in[t].rearrange("j p i -> p j i"),
                ofin[:, 512 * t : 512 * (t + 1)],
            )
            if t > 0:
                recv(t - 1)
            collective(a2a_in[t], a2a_out[t])

        for tb in range(4):
            vtrans(tb, kvsbs[tb])
        qproj(1)
        nflush = attn_qb(0, 0)
        qproj(2)
        nflush = attn_qb(0, 1, flush=nflush)
        nflush()
        exchange(0)
        nflush = attn_qb(1, 0)
        qproj(3)
        nflush = attn_qb(1, 1, flush=nflush)
        nflush()
        exchange(1)
        nflush = attn_qb(2, 0)
        nflush = attn_qb(2, 1, flush=nflush)
        nflush()
        exchange(2)
        # round 3, head by head; first output-projection half in between
        nflush = attn_qb(3, 0)
        nflush()
        nc.sync.dma_start(
            a2a_in3[0].rearrange("j p i -> p j i"), ofin[0:64, 1536:2048]
        )
        recv(2)
        collective(a2a_in3[0], a2a_out3[0])
        oproj_half(0, og_a)
        nflush = attn_qb(3, 1)
        nflush()
        nc.sync.dma_start(
            a2a_in3[1].rearrange("j p i -> p j i"), ofin[64:128, 1536:2048]
        )
        nc.scalar.dma_start(
            og_b[0:64, :, 64:128], a2a_out3[0].rearrange("c p i -> p c i")
        )
        collective(a2a_in3[1], a2a_out3[1])
        # output projection for tokens 128:256, contraction split by peer-row
        # half: the head0-rows half depends only on cc#2 + #3a and runs while
        # #3b is still in flight; only the head1-rows half waits for #3b.
        po_f = []
        for n_ in range(2):
            po = ps_small.tile([128, 512], F32, tag="small", name=f"po_f{n_}")
            for c in range(NCHUNK):
                nc.tensor.matmul(
                    po[:],
                    og_b[0:64, c, :],
                    wo_sb[0:64, c, n_ * 512 : (n_ + 1) * 512],
                    start=(c == 0), stop=False,
                )
            po_f.append(po)
        nc.scalar.dma_start(
            og_b[64:128, :, 64:128], a2a_out3[1].rearrange("c p i -> p c i")
        )
        for n_ in range(2):
            for c in range(NCHUNK):
                nc.tensor.matmul(
                    po_f[n_][:],
                    og_b[64:128, c, :],
                    wo_sb[64:128, c, n_ * 512 : (n_ + 1) * 512],
                    start=False, stop=(c == NCHUNK - 1),
                )
            osb = wp.tile([128, 512], F32, tag="osb")
            nc.scalar.copy(osb[:], po_f[n_][:])
            nc.sync.dma_start(
                out[128:256, n_ * 512 : (n_ + 1) * 512], osb[:]
            )


_NC_CACHE = None


def _get_nc():
    global _NC_CACHE
    if _NC_CACHE is None:
        _NC_CACHE = _build_nc()
    return _NC_CACHE


# ---------------------------------------------------------------- host wrapper
_ROPE_PERM = np.concatenate([np.arange(0, HD, 2), np.arange(1, HD, 2)])


def _chunked(w):
    """(D, F) -> (128, D//128, F) so [p, c, f] = w[128c+p, f]."""
    return np.ascontiguousarray(
        w.reshape(D // 128, 128, -1).transpose(1, 0, 2)
    )


def _prep_inputs(x, freqs_cos, freqs_sin, Wq, Wk, Wv, Wo):
    x = np.asarray(x, np.float32)
    Wq = np.asarray(Wq, np.float32)
    Wk = np.asarray(Wk, np.float32)
    Wv = np.asarray(Wv, np.float32)
    Wo = np.asarray(Wo, np.float32)
    cos = np.asarray(freqs_cos, np.float32)
    sin = np.asarray(freqs_sin, np.float32)

    # [tb, p, c, t]: element = x[512*tb + t, 128*c + p]; each tb block is one
    # contiguous 1MB HBM read on the device
    xt = np.ascontiguousarray(
        x.reshape(4, 512, NCHUNK, 128).transpose(0, 3, 2, 1)
    ).astype(bf16)
    cosf = np.tile(cos.T, (4, 1)).astype(bf16)
    # signed sin table matching the [real(32); imag(32)] row blocks:
    # q' = q*cos + swap(q)*[-s; +s]
    sinf = np.tile(np.concatenate([-sin.T, sin.T], axis=0), (2, 1)).astype(bf16)
    wo_dev = _chunked(Wo).astype(bf16)

    in_maps = []
    for r in range(NC):
        h0, h1 = 2 * r, 2 * r + 1
        g = r // 2
        # q pre-scaled by 1/32: folds the 1/sqrt(HD)=1/8 softmax scale and the
        # /4 for the (cubic)^4 exp decomposition into the weights.
        wq_core = np.concatenate(
            [
                Wq[:, 64 * h0 + _ROPE_PERM],
                Wq[:, 64 * h1 + _ROPE_PERM],
            ],
            axis=1,
        ) * (1.0 / 32.0)
        wkv_core = np.concatenate(
            [Wk[:, 64 * g + _ROPE_PERM], Wv[:, 64 * g : 64 * g + HD]], axis=1
        )
        in_maps.append(
            {
                "xt": xt,
                "wq": _chunked(wq_core).astype(bf16),
                "wkv": _chunked(wkv_core).astype(bf16),
                "wo": wo_dev,
                "cosf": cosf,
                "sinf": sinf,
            }
        )
    return in_maps


def _run(inputs, trace=False, **spmd_kwargs):
    in_maps = _prep_inputs(**inputs)
    nc = _get_nc()
    res = run_bass_kernel_spmd(
        nc, in_maps, core_ids=list(range(NC)), trace=trace, **spmd_kwargs
    )
    # core j's local row (64t + i) is global token (512t + 64j + i)
    allres = np.stack([res.results[r]["out"] for r in range(NC)])  # [8, 256, D]
    full = (
        allres.reshape(NC, 4, 64, D).transpose(1, 0, 2, 3).reshape(N, D)
    )
    return full.astype(np.float32), res


def kernel(**inputs):
    out, _ = _run(inputs, trace=False)
    return out
